# revision 41
# baseline (speedup 1.0000x reference)
# Trainium2 Bass kernel for nn_Actor: 2-layer LSTM actor head.
# B=8192 sharded 1024/core across 8 NeuronCores (pure data parallel,
# replicated weights, no collectives). Feature-major layout: all [H, B]
# tensors stored as two [128, 1024] partition tiles. Matmuls in bf16
# (N=512 moving chunks, f32 PSUM accumulate).
#
# Per step t (PyTorch gate order i,f,g,o):
#   cell0 gates = W_hh0 @ h0 + onehot(prev-action) @ V + pre0
#     pre0  = W_ih0[:, :256] @ relu(se_W @ states.T + se_b)   (precomputed)
#     V[j]  = W_ih0[:, 256:] @ aemb[j] + b_ih0 + b_hh0        (K=3 matmul)
#   cell1 gates = W_ih1 @ h0n + W_hh1 @ h1  (+bias via ACT)
#   z_t = (hW[:,1]-hW[:,0]) . h1n   -> accumulated into a pinned PSUM
#         [32, 1024] tile via zero-padded M=32 stationaries.
# Post-loop: logp/entropy from z (softplus/sigmoid), prefix sums via
# strict-lower-triangular matmul.
import sys

if "/opt/trn_rl_repo" not in sys.path and "/root/.axon_site/_ro/trn_rl_repo" not in sys.path:
    sys.path.insert(0, "/opt/trn_rl_repo")

from contextlib import ExitStack

import ml_dtypes
import numpy as np

import bass_rust
import concourse.bass as bass
import concourse.tile as tile
from concourse import mybir
from concourse.bass_utils import run_bass_kernel_spmd

BF = ml_dtypes.bfloat16
AF = mybir.ActivationFunctionType
DT_BF = mybir.dt.bfloat16
DT_F32 = mybir.dt.float32
DT_F8 = mybir.dt.float8e4
NP_F8 = mybir.dt.np(mybir.dt.float8e4)

B, S, D, H, E = 8192, 256, 32, 256, 32
NCORES = 8
BL = B // NCORES  # 1024 per core
N5 = 512  # moving free-dim chunk

# wstack column layout (all bf16, [128, WCOLS])
SE = 0            # se_W.T          4  tiles [128,128] (k*2+m)
IH0 = 512         # W_ih0[:,:256].T 16 tiles (k*8+m)
HH0 = IH0 + 2048  # W_hh0.T         16 tiles
IH1 = HH0 + 2048  # W_ih1.T         16 tiles
HH1 = IH1 + 2048  # W_hh1.T         16 tiles
OHW = HH1 + 2048  # [3, 1024] one-hot stationary rows (v0,v1,v2)
DWO = OHW + 1024  # 64 blocks [128, 32]: (t*2+k) -> col t holds dW_t chunk k
TRI = DWO + 2048  # [32, 34]: strict-lower-tri(32) | ones | ones
IDN = TRI + 34    # [128, 128] identity (pre0 PSUM injection)
WCOLS = IDN + 128

# fp8 wstack8 (all values pre-scaled x4; DoubleRow K=256 contractions).
# Layout per m-tile: [k0-block 128 | k1-block 128] (m-major).
H0D = 0
I1D = 2048
H1D = 4096
DW8 = 6144        # per t: [k0 32 | k1 32]
W8COLS = 8192
FP8_SCALE = 1.0
DESCALE = 1.0 / (FP8_SCALE * FP8_SCALE)

MAX_WAITS = 1


def _split_multi_waits(nc):
    # This compiler build encodes only one sync wait per instruction; move
    # excess waits onto InstNoOp spliced directly before (same engine queue).
    for f in nc.m.functions:
        for bb in f.blocks:
            out, changed = [], False
            for inst in bb.instructions:
                si = inst.sync_info
                waits = list(si.on_wait or []) if si else []
                if len(waits) > MAX_WAITS:
                    changed = True
                    extra, keep = waits[MAX_WAITS:], waits[:MAX_WAITS]
                    for j in range(0, len(extra), MAX_WAITS):
                        nop = mybir.InstNoOp(
                            name=nc.get_next_instruction_name(),
                            engine=inst.engine,
                            sync_info=mybir.SyncInfo(
                                on_wait=extra[j : j + MAX_WAITS], on_update=[]
                            ),
                            bass_nofuse=True,
                            text_hint="split_wait",
                        )
                        nc.register_instruction(nop)
                        out.append(nop)
                    si.on_wait = keep
                    inst.sync_info = si
                out.append(inst)
            if changed:
                bb.instructions = out


def _patch_drain_and_barrier():
    from concourse.tile import ScopedClock

    def _drain_and_barrier(self, tick_clock, wait_clock):
        nc = self.nc
        spares = [nc.sync.nop(nofuse=True, hint=f"dw{i}") for i in range(32)]
        drain_inst = nc.sync.drain()
        wait_clock.add_sem_waits(
            drain_inst.ins, ScopedClock({None: tick_clock.global_clock})
        )
        si = drain_inst.ins.sync_info
        waits = list(si.on_wait or [])
        if len(waits) > MAX_WAITS:
            extra, keep = waits[MAX_WAITS:], waits[:MAX_WAITS]
            si.on_wait = keep
            drain_inst.ins.sync_info = si
            for i in range(0, len(extra), MAX_WAITS):
                idx = i // MAX_WAITS
                assert idx < len(spares)
                spares[idx].ins.sync_info = bass_rust.SyncInfo(
                    on_wait=extra[i : i + MAX_WAITS], on_update=[]
                )
        nc.all_engine_barrier()
        assert self.sems is not None
        popped = nc._tile_sem_poison_stack.pop()
        assert popped is self._sem_poison
        nc.clear_and_free_semaphores(list(self.sems.allocated().values()))
        nc.all_engine_barrier()

    tile.TileContext._drain_and_barrier = _drain_and_barrier


_patch_drain_and_barrier()


def _w(nc, wt, off, k, m, stride):
    # stationary tile [128, 128] at region `off`, K-tile k, M-tile m
    c = off + (k * stride + m) * 128
    return wt[:, c : c + 128]


def build_nc():
    nc = bass.Bass()
    wstack = nc.declare_dram_parameter("wstack", [128, WCOLS], DT_BF, isOutput=False)
    wstack8 = nc.declare_dram_parameter("wstack8", [128, W8COLS], DT_F8, isOutput=False)
    misc = nc.declare_dram_parameter("misc", [128, 16], DT_F32, isOutput=False)
    sT = nc.declare_dram_parameter("sT", [128, 2 * BL], DT_BF, isOutput=False)
    ohd = nc.declare_dram_parameter("ohd", [3 * D, BL], DT_BF, isOutput=False)
    a1 = nc.declare_dram_parameter("a1", [D, BL], DT_F32, isOutput=False)
    out = nc.declare_dram_parameter("out", [2 * D + 2, BL], DT_F32, isOutput=True)

    with tile.TileContext(nc) as tc, ExitStack() as ctx:
        const = ctx.enter_context(tc.tile_pool(name="const", bufs=1))
        ps = ctx.enter_context(tc.tile_pool(name="ps", bufs=3, space="PSUM"))
        zpsp = ctx.enter_context(tc.tile_pool(name="zps", bufs=1, space="PSUM"))
        # loop-scoped SBUF pools: released before the epilogue pool opens
        lctx = ctx.enter_context(ExitStack())
        state = lctx.enter_context(tc.tile_pool(name="state", bufs=2))
        gates = lctx.enter_context(tc.tile_pool(name="gates", bufs=3))
        tmp = lctx.enter_context(tc.tile_pool(name="tmp", bufs=2))
        prep = lctx.enter_context(tc.tile_pool(name="pre", bufs=1))
        stgp = lctx.enter_context(tc.tile_pool(name="stg", bufs=4))

        wt = const.tile([128, WCOLS], DT_BF, tag="wt")
        # prologue weights (SE+IH0) first so s_emb/pre0 matmuls start early
        nc.sync.dma_start(wt[:, 0 : HH0], wstack[:, 0 : HH0])
        msc = const.tile([128, 16], DT_F32, tag="msc")
        nc.gpsimd.dma_start(msc[:], misc[:])
        wt8 = const.tile([128, W8COLS], DT_F8, tag="wt8")

        pre0 = prep.tile([128, 8 * BL], DT_BF, tag="pre0")
        zps = zpsp.tile([32, BL], DT_F32, tag="zt")

        # ---------- prologue: s_emb, pre0 ----------
        with tc.tile_pool(name="pro", bufs=1) as prol:
            st = prol.tile([128, 2 * BL], DT_BF, tag="st")
            nc.gpsimd.dma_start(st[:], sT[:])
            nc.gpsimd.dma_start(wt8[:], wstack8[:])
            nc.sync.dma_start(wt[:, HH0:], wstack[:, HH0:])
            semb = prol.tile([128, 2 * BL], DT_BF, tag="semb")
            for m2 in range(2):
                pt = ps.tile([128, BL], DT_F32, tag="g")
                for k in range(2):
                    for n in range(2):
                        nc.tensor.matmul(
                            pt[:, n * N5 : (n + 1) * N5],
                            _w(nc, wt, SE, k, m2, 2),
                            st[:, k * BL + n * N5 : k * BL + (n + 1) * N5],
                            start=(k == 0),
                            stop=(k == 1),
                        )
                nc.scalar.activation(
                    semb[:, m2 * BL : (m2 + 1) * BL], pt[:], AF.Relu,
                    bias=msc[:, m2 : m2 + 1],
                )
            for m in range(8):
                pt = ps.tile([128, BL], DT_F32, tag="g")
                for k in range(2):
                    for n in range(2):
                        nc.tensor.matmul(
                            pt[:, n * N5 : (n + 1) * N5],
                            _w(nc, wt, IH0, k, m, 8),
                            semb[:, k * BL + n * N5 : k * BL + (n + 1) * N5],
                            start=(k == 0),
                            stop=(k == 1),
                        )
                nc.scalar.activation(pre0[:, m * BL : (m + 1) * BL], pt[:], AF.Copy)

        # ---------- init state ----------
        # states stored flat [128, 2*BL]: cols 0:BL = features 0-127,
        # cols BL:2BL = features 128-255 (k-tile halves for matmul moving)
        st_tiles = {}
        for nm in ("h0", "c0", "h1", "c1"):
            dt_ = DT_F8 if nm.startswith("h") else DT_BF
            t_ = state.tile([128, 2 * BL], dt_, tag=nm, name=f"st_{nm}")
            nc.gpsimd.memset(t_[:], 0.0)
            st_tiles[nm] = t_

        def mv3(ht, n):
            # DoubleRow moving operand [128, 2, 512]: k-halves as blocks
            return ht[:].rearrange("p (k j) -> p k j", k=2)[
                :, :, n * N5 : (n + 1) * N5
            ]

        def w3(off, m, q=128):
            c = off + m * 2 * q
            return wt8[:, c : c + 2 * q].rearrange("p (k q) -> p k q", k=2)

        def mv(tile_, k, n):
            # moving chunk: k-tile half k, batch chunk n
            return tile_[:, k * BL + n * N5 : k * BL + (n + 1) * N5]

        MORDER = [0, 2, 4, 6, 1, 3, 5, 7]

        def update_c(cell, kh, G, cn):
            # cn = f*c + i*tanh(g) needs only the i/f/g gates (not o) so it
            # can start one m-tile earlier than the o-gate ACT
            sl = slice(kh * BL, (kh + 1) * BL)
            c_old = st_tiles[f"c{cell}"]
            m1 = tmp.tile([128, BL], DT_BF, tag="m1", name="m1")
            nc.vector.tensor_mul(m1[:], G[1][:, sl], c_old[:, sl])
            m2 = tmp.tile([128, BL], DT_BF, tag="m2", name="m2")
            nc.vector.tensor_mul(m2[:], G[0][:, sl], G[2][:, sl])
            nc.vector.tensor_add(cn[:, sl], m1[:], m2[:])

        def update_h(cell, kh, G, cn, hn):
            sl = slice(kh * BL, (kh + 1) * BL)
            tc_ = tmp.tile([128, BL], DT_BF, tag=f"tc{kh}", name="tcn")
            nc.scalar.activation(tc_[:], cn[:, sl], AF.Tanh)
            # hn split per batch chunk: downstream DoubleRow matmuls need both
            # feature halves but only one 512-col chunk at a time
            for n in range(2):
                cs = slice(kh * BL + n * N5, kh * BL + (n + 1) * N5)
                nc.vector.tensor_mul(hn[:, cs], G[3][:, cs], tc_[:, n * N5 : (n + 1) * N5])

        def emit_z(t, h1t):
            for n in range(2):
                nc.tensor.matmul(
                    zps[:, n * N5 : (n + 1) * N5],
                    w3(DW8, t, q=32),
                    mv3(h1t, n),
                    start=(t == 0),
                    stop=(t == D - 1),
                    skip_group_check=True,
                    perf_mode=mybir.MatmulPerfMode.DoubleRow,
                )

        # ---------- recurrent loop ----------
        prev_z = None  # deferred z so PE never waits on h1n
        for t in range(D):
            stg = stgp.tile([3, BL], DT_BF, tag="stg", name="stg")
            nc.sync.dma_start(stg[:], ohd[3 * t : 3 * t + 3, :])

            # ----- cell 0: group = [pre0-inject, onehot, hh0 x4]; ACT drains.
            # Half-A m-tiles first so h0n half A is ready while PE runs half B.
            h0t = st_tiles["h0"]
            G0 = [None] * 4
            for g in range(4):
                G0[g] = gates.tile([128, 2 * BL], DT_BF, tag=f"g{g}", name=f"g0_{g}")
            c0n = state.tile([128, 2 * BL], DT_BF, tag="c0", name="c0n")
            h0nt = state.tile([128, 2 * BL], DT_F8, tag="h0", name="h0nt")
            for mi, m in enumerate(MORDER):
                g, hf = m // 2, m % 2
                pt = ps.tile([128, BL], DT_F32, tag="g")
                for n in range(2):
                    nc.tensor.matmul(
                        pt[:, n * N5 : (n + 1) * N5],
                        wt[:, IDN : IDN + 128],
                        pre0[:, m * BL + n * N5 : m * BL + (n + 1) * N5],
                        start=True,
                        stop=False,
                    )
                for n in range(2):
                    nc.tensor.matmul(
                        pt[:, n * N5 : (n + 1) * N5],
                        wt[0:3, OHW + m * 128 : OHW + (m + 1) * 128],
                        stg[0:3, n * N5 : (n + 1) * N5],
                        start=False,
                        stop=False,
                    )
                for n in range(2):
                    nc.tensor.matmul(
                        pt[:, n * N5 : (n + 1) * N5],
                        w3(H0D, m),
                        mv3(h0t, n),
                        start=False,
                        stop=True,
                        perf_mode=mybir.MatmulPerfMode.DoubleRow,
                    )
                nc.scalar.activation(
                    G0[g][:, hf * BL : (hf + 1) * BL], pt[:],
                    AF.Tanh if g == 2 else AF.Sigmoid, scale=DESCALE,
                )
                if mi == 1 and prev_z is not None:
                    emit_z(*prev_z)
                if mi == 2:
                    update_c(0, 0, G0, c0n)
                elif mi == 3:
                    update_h(0, 0, G0, c0n, h0nt)
                elif mi == 6:
                    update_c(0, 1, G0, c0n)
                elif mi == 7:
                    update_h(0, 1, G0, c0n, h0nt)
            st_tiles["c0"] = c0n
            st_tiles["h0"] = h0nt

            # ----- cell 1: hh1 (old h1) first, then ih1 (fresh h0n)
            h0n, h1t = st_tiles["h0"], st_tiles["h1"]
            G1 = [None] * 4
            for g in range(4):
                G1[g] = gates.tile([128, 2 * BL], DT_BF, tag=f"g{g}", name=f"g1_{g}")
            c1n = state.tile([128, 2 * BL], DT_BF, tag="c1", name="c1n")
            h1nt = state.tile([128, 2 * BL], DT_F8, tag="h1", name="h1nt")
            for mi, m in enumerate(MORDER):
                g, hf = m // 2, m % 2
                pt = ps.tile([128, BL], DT_F32, tag="g")
                for n in range(2):
                    nc.tensor.matmul(
                        pt[:, n * N5 : (n + 1) * N5],
                        w3(H1D, m),
                        mv3(h1t, n),
                        start=True,
                        stop=False,
                        perf_mode=mybir.MatmulPerfMode.DoubleRow,
                    )
                for n in range(2):
                    nc.tensor.matmul(
                        pt[:, n * N5 : (n + 1) * N5],
                        w3(I1D, m),
                        mv3(h0n, n),
                        start=False,
                        stop=True,
                        perf_mode=mybir.MatmulPerfMode.DoubleRow,
                    )
                nc.scalar.activation(
                    G1[g][:, hf * BL : (hf + 1) * BL], pt[:],
                    AF.Tanh if g == 2 else AF.Sigmoid,
                    bias=msc[:, 2 + m : 3 + m], scale=DESCALE,
                )
                if mi == 2:
                    update_c(1, 0, G1, c1n)
                elif mi == 3:
                    update_h(1, 0, G1, c1n, h1nt)
                elif mi == 6:
                    update_c(1, 1, G1, c1n)
                elif mi == 7:
                    update_h(1, 1, G1, c1n, h1nt)
            st_tiles["c1"] = c1n
            st_tiles["h1"] = h1nt
            prev_z = (t, st_tiles["h1"])

        emit_z(*prev_z)
        lctx.close()  # free loop SBUF before the epilogue pool allocates

        # ---------- epilogue ----------
        with tc.tile_pool(name="ep", bufs=1) as ep:
            a1t = ep.tile([D, BL], DT_F32, tag="a1t")
            nc.gpsimd.dma_start(a1t[:], a1[:])
            z = ep.tile([D, BL], DT_F32, tag="z")
            ez = ep.tile([D, BL], DT_F32, tag="ez")
            fz = ep.tile([D, BL], DT_F32, tag="fz")
            sp = ep.tile([D, BL], DT_F32, tag="sp")
            rf = ep.tile([D, BL], DT_F32, tag="rf")
            sg = ep.tile([D, BL], DT_F32, tag="sg")
            zsg = ep.tile([D, BL], DT_F32, tag="zsg")
            pe_f = ep.tile([D, BL], DT_F32, tag="pe_f")
            pe_b = ep.tile([D, BL], DT_BF, tag="pe_b")
            az = ep.tile([D, BL], DT_F32, tag="az")
            lp_b = ep.tile([D, BL], DT_BF, tag="lp_b")
            res = ep.tile([33, BL], DT_F32, tag="res")
            res2 = ep.tile([1, BL], DT_F32, tag="res2")
            for n in range(2):
                cs = slice(n * N5, (n + 1) * N5)
                # softplus(z) = ln(1+exp(z)); sigmoid(z) = exp(z)/(1+exp(z))
                if DESCALE == 1.0:
                    nc.vector.tensor_scalar_add(z[:, cs], zps[:, cs], msc[0:D, 10:11])
                else:
                    nc.vector.tensor_scalar(
                        z[:, cs], zps[:, cs], DESCALE, msc[0:D, 10:11],
                        mybir.AluOpType.mult, mybir.AluOpType.add,
                    )
                nc.scalar.activation(ez[:, cs], z[:, cs], AF.Exp)
                nc.scalar.add(fz[:, cs], ez[:, cs], 1.0)
                nc.scalar.activation(sp[:, cs], fz[:, cs], AF.Ln)
                nc.vector.reciprocal(rf[:, cs], fz[:, cs])
                nc.vector.tensor_mul(sg[:, cs], ez[:, cs], rf[:, cs])
                nc.vector.tensor_mul(zsg[:, cs], z[:, cs], sg[:, cs])
                nc.vector.tensor_sub(pe_f[:, cs], sp[:, cs], zsg[:, cs])
                nc.vector.tensor_copy(pe_b[:, cs], pe_f[:, cs])
                nc.vector.tensor_mul(az[:, cs], a1t[:, cs], z[:, cs])
                nc.vector.tensor_sub(lp_b[:, cs], az[:, cs], sp[:, cs])
                ptp = ps.tile([128, BL], DT_F32, tag="g", name=f"ptp{n}")
                nc.tensor.matmul(
                    ptp[0:33, 0:N5],
                    wt[0:32, TRI : TRI + 33],
                    lp_b[:, cs],
                    start=True,
                    stop=True,
                )
                nc.scalar.activation(res[0:33, cs], ptp[0:33, 0:N5], AF.Copy)
                pte = ps.tile([128, BL], DT_F32, tag="g", name=f"pte{n}")
                nc.tensor.matmul(
                    pte[0:1, 0:N5],
                    wt[0:32, TRI + 33 : TRI + 34],
                    pe_b[:, cs],
                    start=True,
                    stop=True,
                )
                nc.scalar.activation(res2[0:1, cs], pte[0:1, 0:N5], AF.Copy)

            nc.sync.dma_start(out[0:D, :], res[0:D, :])          # prefix_lps.T
            nc.sync.dma_start(out[D : 2 * D, :], pe_f[:])        # per_ent.T
            nc.sync.dma_start(out[2 * D : 2 * D + 1, :], res[D : D + 1, :])  # tot_lp
            nc.sync.dma_start(out[2 * D + 1 : 2 * D + 2, :], res2[0:1, :])   # tot_ent

    _split_multi_waits(nc)
    return nc


def prep_inputs(inputs):
    f32 = np.float32
    states = np.asarray(inputs["states"], f32)
    actions = np.asarray(inputs["actions"]).astype(np.int32)
    se_W = np.asarray(inputs["se_W"], f32)
    se_b = np.asarray(inputs["se_b"], f32)
    aemb = np.asarray(inputs["aemb"], f32)
    W_ih0 = np.asarray(inputs["W_ih0"], f32)
    W_hh0 = np.asarray(inputs["W_hh0"], f32)
    b_ih0 = np.asarray(inputs["b_ih0"], f32)
    b_hh0 = np.asarray(inputs["b_hh0"], f32)
    W_ih1 = np.asarray(inputs["W_ih1"], f32)
    W_hh1 = np.asarray(inputs["W_hh1"], f32)
    b_ih1 = np.asarray(inputs["b_ih1"], f32)
    b_hh1 = np.asarray(inputs["b_hh1"], f32)
    heads_W = np.asarray(inputs["heads_W"], f32)
    heads_b = np.asarray(inputs["heads_b"], f32)

    W = np.zeros((128, WCOLS), f32)

    def put(off, kt, mt, stride, mat):
        for k in range(kt):
            for m in range(mt):
                c = off + (k * stride + m) * 128
                W[:, c : c + 128] = mat[k * 128 : (k + 1) * 128, m * 128 : (m + 1) * 128]

    put(SE, 2, 2, 2, se_W.T)
    # IH0 scaled x16: pre0 lives at 16x so it matches the fp8 (4x * 4x)
    # scaled hh0 contributions; gate ACT descales by 1/16.
    put(IH0, 2, 8, 8, (FP8_SCALE * FP8_SCALE) * W_ih0[:, :H].T)
    # one-hot stationary rows (x16): v_j = W_ih0[:, H:] @ aemb[j] + b0
    vs = aemb @ W_ih0[:, H:].T + (b_ih0 + b_hh0)[None, :]  # [3, 1024]
    W[0:3, OHW : OHW + 1024] = (FP8_SCALE * FP8_SCALE) * vs
    W[0:32, TRI : TRI + 32] = np.triu(np.ones((32, 32), f32), k=1)
    W[0:32, TRI + 32] = 1.0
    W[0:32, TRI + 33] = 1.0
    W[:, IDN : IDN + 128] = np.eye(128, dtype=f32)
    wstack = W.astype(BF)

    # fp8 stack: x4-scaled weights, m-major [k0 | k1] blocks per m-tile
    W8 = np.zeros((128, W8COLS), f32)

    def put8(off, mat, q=128):
        # mat [256, M]: m-tile blocks [k0 128cols | k1 128cols]
        mt = mat.shape[1] // q
        for m in range(mt):
            for k in range(2):
                c = off + m * 2 * q + k * q
                W8[:, c : c + q] = mat[k * 128 : (k + 1) * 128, m * q : (m + 1) * q]

    put8(H0D, FP8_SCALE * W_hh0.T)
    put8(I1D, FP8_SCALE * W_ih1.T)
    put8(H1D, FP8_SCALE * W_hh1.T)
    dW = heads_W[:, :, 1] - heads_W[:, :, 0]  # [32, 256]
    dstack = np.zeros((256, D * 32), f32)
    for t in range(D):
        dstack[:, t * 32 + t] = FP8_SCALE * dW[t]
    put8(DW8, dstack, q=32)
    wstack8 = W8.astype(NP_F8)

    misc = np.zeros((128, 16), f32)
    misc[:, 0] = se_b[:128]
    misc[:, 1] = se_b[128:]
    b1 = b_ih1 + b_hh1
    for m in range(8):
        misc[:, 2 + m] = b1[m * 128 : (m + 1) * 128]
    misc[0:D, 10] = heads_b[:, 1] - heads_b[:, 0]

    in_maps = []
    for c in range(NCORES):
        sl = slice(c * BL, (c + 1) * BL)
        st = states[sl].T  # [256, 1024]
        sTv = np.concatenate([st[:128], st[128:]], axis=1).astype(BF)
        oh = np.zeros((3 * D, BL), f32)
        oh[2, :] = 1.0  # t=0: start token row
        for t in range(1, D):
            ap = actions[sl, t - 1].astype(f32)
            oh[3 * t + 0] = 1.0 - ap
            oh[3 * t + 1] = ap
        a1v = actions[sl].T.astype(f32)
        in_maps.append(
            {
                "wstack": wstack,
                "wstack8": wstack8,
                "misc": misc,
                "sT": sTv,
                "ohd": oh.astype(BF),
                "a1": np.ascontiguousarray(a1v),
            }
        )
    return in_maps


_CACHED_NC = None


def run(inputs, trace=False):
    global _CACHED_NC
    in_maps = prep_inputs(inputs)
    if _CACHED_NC is None:
        _CACHED_NC = build_nc()
    res = None
    for attempt in range(3):
        try:
            res = run_bass_kernel_spmd(
                _CACHED_NC, in_maps, core_ids=list(range(NCORES)), trace=trace
            )
            break
        except Exception:
            if attempt == 2:
                raise
    assert res is not None
    tot_lp = np.empty((B,), np.float32)
    tot_ent = np.empty((B,), np.float32)
    prefix = np.empty((B, D), np.float32)
    per_ent = np.empty((B, D), np.float32)
    for c in range(NCORES):
        o = res.results[c]["out"]
        sl = slice(c * BL, (c + 1) * BL)
        prefix[sl] = o[0:D].T
        per_ent[sl] = o[D : 2 * D].T
        tot_lp[sl] = o[2 * D]
        tot_ent[sl] = o[2 * D + 1]
    return (tot_lp, tot_ent, prefix, per_ent), res


def kernel(**inputs):
    out, _ = run(inputs, trace=False)
    return out


# revision 42
# speedup vs baseline: 1.0033x; 1.0033x over previous
# Trainium2 Bass kernel for nn_Actor: 2-layer LSTM actor head.
# B=8192 sharded 1024/core across 8 NeuronCores (pure data parallel,
# replicated weights, no collectives). Feature-major layout: all [H, B]
# tensors stored as two [128, 1024] partition tiles. Matmuls in bf16
# (N=512 moving chunks, f32 PSUM accumulate).
#
# Per step t (PyTorch gate order i,f,g,o):
#   cell0 gates = W_hh0 @ h0 + onehot(prev-action) @ V + pre0
#     pre0  = W_ih0[:, :256] @ relu(se_W @ states.T + se_b)   (precomputed)
#     V[j]  = W_ih0[:, 256:] @ aemb[j] + b_ih0 + b_hh0        (K=3 matmul)
#   cell1 gates = W_ih1 @ h0n + W_hh1 @ h1  (+bias via ACT)
#   z_t = (hW[:,1]-hW[:,0]) . h1n   -> accumulated into a pinned PSUM
#         [32, 1024] tile via zero-padded M=32 stationaries.
# Post-loop: logp/entropy from z (softplus/sigmoid), prefix sums via
# strict-lower-triangular matmul.
import sys

if "/opt/trn_rl_repo" not in sys.path and "/root/.axon_site/_ro/trn_rl_repo" not in sys.path:
    sys.path.insert(0, "/opt/trn_rl_repo")

from contextlib import ExitStack

import ml_dtypes
import numpy as np

import bass_rust
import concourse.bass as bass
import concourse.tile as tile
from concourse import mybir
from concourse.bass_utils import run_bass_kernel_spmd

BF = ml_dtypes.bfloat16
AF = mybir.ActivationFunctionType
DT_BF = mybir.dt.bfloat16
DT_F32 = mybir.dt.float32
DT_F8 = mybir.dt.float8e4
NP_F8 = mybir.dt.np(mybir.dt.float8e4)

B, S, D, H, E = 8192, 256, 32, 256, 32
NCORES = 8
BL = B // NCORES  # 1024 per core
N5 = 512  # moving free-dim chunk

# wstack column layout (all bf16, [128, WCOLS])
SE = 0            # se_W.T          4  tiles [128,128] (k*2+m)
IH0 = 512         # W_ih0[:,:256].T 16 tiles (k*8+m)
HH0 = IH0 + 2048  # W_hh0.T         16 tiles
IH1 = HH0 + 2048  # W_ih1.T         16 tiles
HH1 = IH1 + 2048  # W_hh1.T         16 tiles
OHW = HH1 + 2048  # [3, 1024] one-hot stationary rows (v0,v1,v2)
DWO = OHW + 1024  # 64 blocks [128, 32]: (t*2+k) -> col t holds dW_t chunk k
TRI = DWO + 2048  # [32, 34]: strict-lower-tri(32) | ones | ones
IDN = TRI + 34    # [128, 128] identity (pre0 PSUM injection)
WCOLS = IDN + 128

# fp8 wstack8 (all values pre-scaled x4; DoubleRow K=256 contractions).
# Layout per m-tile: [k0-block 128 | k1-block 128] (m-major).
H0D = 0
I1D = 2048
H1D = 4096
DW8 = 6144        # per t: [k0 32 | k1 32]
W8COLS = 8192
FP8_SCALE = 1.0
DESCALE = 1.0 / (FP8_SCALE * FP8_SCALE)

MAX_WAITS = 1


def _split_multi_waits(nc):
    # This compiler build encodes only one sync wait per instruction; move
    # excess waits onto InstNoOp spliced directly before (same engine queue).
    for f in nc.m.functions:
        for bb in f.blocks:
            out, changed = [], False
            for inst in bb.instructions:
                si = inst.sync_info
                waits = list(si.on_wait or []) if si else []
                if len(waits) > MAX_WAITS:
                    changed = True
                    extra, keep = waits[MAX_WAITS:], waits[:MAX_WAITS]
                    for j in range(0, len(extra), MAX_WAITS):
                        nop = mybir.InstNoOp(
                            name=nc.get_next_instruction_name(),
                            engine=inst.engine,
                            sync_info=mybir.SyncInfo(
                                on_wait=extra[j : j + MAX_WAITS], on_update=[]
                            ),
                            bass_nofuse=True,
                            text_hint="split_wait",
                        )
                        nc.register_instruction(nop)
                        out.append(nop)
                    si.on_wait = keep
                    inst.sync_info = si
                out.append(inst)
            if changed:
                bb.instructions = out


def _patch_drain_and_barrier():
    from concourse.tile import ScopedClock

    def _drain_and_barrier(self, tick_clock, wait_clock):
        nc = self.nc
        spares = [nc.sync.nop(nofuse=True, hint=f"dw{i}") for i in range(32)]
        drain_inst = nc.sync.drain()
        wait_clock.add_sem_waits(
            drain_inst.ins, ScopedClock({None: tick_clock.global_clock})
        )
        si = drain_inst.ins.sync_info
        waits = list(si.on_wait or [])
        if len(waits) > MAX_WAITS:
            extra, keep = waits[MAX_WAITS:], waits[:MAX_WAITS]
            si.on_wait = keep
            drain_inst.ins.sync_info = si
            for i in range(0, len(extra), MAX_WAITS):
                idx = i // MAX_WAITS
                assert idx < len(spares)
                spares[idx].ins.sync_info = bass_rust.SyncInfo(
                    on_wait=extra[i : i + MAX_WAITS], on_update=[]
                )
        nc.all_engine_barrier()
        assert self.sems is not None
        popped = nc._tile_sem_poison_stack.pop()
        assert popped is self._sem_poison
        nc.clear_and_free_semaphores(list(self.sems.allocated().values()))
        nc.all_engine_barrier()

    tile.TileContext._drain_and_barrier = _drain_and_barrier


_patch_drain_and_barrier()


def _w(nc, wt, off, k, m, stride):
    # stationary tile [128, 128] at region `off`, K-tile k, M-tile m
    c = off + (k * stride + m) * 128
    return wt[:, c : c + 128]


def build_nc():
    nc = bass.Bass()
    wstack = nc.declare_dram_parameter("wstack", [128, WCOLS], DT_BF, isOutput=False)
    wstack8 = nc.declare_dram_parameter("wstack8", [128, W8COLS], DT_F8, isOutput=False)
    misc = nc.declare_dram_parameter("misc", [128, 16], DT_F32, isOutput=False)
    sT = nc.declare_dram_parameter("sT", [128, 2 * BL], DT_BF, isOutput=False)
    ohd = nc.declare_dram_parameter("ohd", [3 * D, BL], DT_BF, isOutput=False)
    a1 = nc.declare_dram_parameter("a1", [D, BL], DT_F32, isOutput=False)
    out = nc.declare_dram_parameter("out", [2 * D + 2, BL], DT_F32, isOutput=True)

    with tile.TileContext(nc) as tc, ExitStack() as ctx:
        const = ctx.enter_context(tc.tile_pool(name="const", bufs=1))
        ps = ctx.enter_context(tc.tile_pool(name="ps", bufs=3, space="PSUM"))
        zpsp = ctx.enter_context(tc.tile_pool(name="zps", bufs=1, space="PSUM"))
        # loop-scoped SBUF pools: released before the epilogue pool opens
        lctx = ctx.enter_context(ExitStack())
        state = lctx.enter_context(tc.tile_pool(name="state", bufs=2))
        gates = lctx.enter_context(tc.tile_pool(name="gates", bufs=3))
        tmp = lctx.enter_context(tc.tile_pool(name="tmp", bufs=2))
        prep = lctx.enter_context(tc.tile_pool(name="pre", bufs=1))
        stgp = lctx.enter_context(tc.tile_pool(name="stg", bufs=4))

        wt = const.tile([128, WCOLS], DT_BF, tag="wt")
        # smallest-first weight DMAs: s_emb needs only SE (128KB); pre0 needs
        # IH0; the rest streams in behind the prologue compute
        nc.sync.dma_start(wt[:, 0 : IH0], wstack[:, 0 : IH0])
        nc.sync.dma_start(wt[:, IH0 : HH0], wstack[:, IH0 : HH0])
        msc = const.tile([128, 16], DT_F32, tag="msc")
        nc.gpsimd.dma_start(msc[:], misc[:])
        wt8 = const.tile([128, W8COLS], DT_F8, tag="wt8")

        pre0 = prep.tile([128, 8 * BL], DT_BF, tag="pre0")
        zps = zpsp.tile([32, BL], DT_F32, tag="zt")

        # ---------- prologue: s_emb, pre0 ----------
        with tc.tile_pool(name="pro", bufs=1) as prol:
            st = prol.tile([128, 2 * BL], DT_BF, tag="st")
            nc.gpsimd.dma_start(st[:], sT[:])
            nc.gpsimd.dma_start(wt8[:], wstack8[:])
            nc.sync.dma_start(wt[:, HH0:], wstack[:, HH0:])
            semb = prol.tile([128, 2 * BL], DT_BF, tag="semb")
            for m2 in range(2):
                pt = ps.tile([128, BL], DT_F32, tag="g")
                for k in range(2):
                    for n in range(2):
                        nc.tensor.matmul(
                            pt[:, n * N5 : (n + 1) * N5],
                            _w(nc, wt, SE, k, m2, 2),
                            st[:, k * BL + n * N5 : k * BL + (n + 1) * N5],
                            start=(k == 0),
                            stop=(k == 1),
                        )
                nc.scalar.activation(
                    semb[:, m2 * BL : (m2 + 1) * BL], pt[:], AF.Relu,
                    bias=msc[:, m2 : m2 + 1],
                )
            for m in range(8):
                pt = ps.tile([128, BL], DT_F32, tag="g")
                for k in range(2):
                    for n in range(2):
                        nc.tensor.matmul(
                            pt[:, n * N5 : (n + 1) * N5],
                            _w(nc, wt, IH0, k, m, 8),
                            semb[:, k * BL + n * N5 : k * BL + (n + 1) * N5],
                            start=(k == 0),
                            stop=(k == 1),
                        )
                nc.scalar.activation(pre0[:, m * BL : (m + 1) * BL], pt[:], AF.Copy)

        # ---------- init state ----------
        # states stored flat [128, 2*BL]: cols 0:BL = features 0-127,
        # cols BL:2BL = features 128-255 (k-tile halves for matmul moving)
        st_tiles = {}
        for nm in ("h0", "c0", "h1", "c1"):
            dt_ = DT_F8 if nm.startswith("h") else DT_BF
            t_ = state.tile([128, 2 * BL], dt_, tag=nm, name=f"st_{nm}")
            nc.gpsimd.memset(t_[:], 0.0)
            st_tiles[nm] = t_

        def mv3(ht, n):
            # DoubleRow moving operand [128, 2, 512]: k-halves as blocks
            return ht[:].rearrange("p (k j) -> p k j", k=2)[
                :, :, n * N5 : (n + 1) * N5
            ]

        def w3(off, m, q=128):
            c = off + m * 2 * q
            return wt8[:, c : c + 2 * q].rearrange("p (k q) -> p k q", k=2)

        def mv(tile_, k, n):
            # moving chunk: k-tile half k, batch chunk n
            return tile_[:, k * BL + n * N5 : k * BL + (n + 1) * N5]

        MORDER = [0, 2, 4, 6, 1, 3, 5, 7]

        def update_c(cell, kh, G, cn):
            # cn = f*c + i*tanh(g) needs only the i/f/g gates (not o) so it
            # can start one m-tile earlier than the o-gate ACT
            sl = slice(kh * BL, (kh + 1) * BL)
            c_old = st_tiles[f"c{cell}"]
            m1 = tmp.tile([128, BL], DT_BF, tag="m1", name="m1")
            nc.vector.tensor_mul(m1[:], G[1][:, sl], c_old[:, sl])
            m2 = tmp.tile([128, BL], DT_BF, tag="m2", name="m2")
            nc.vector.tensor_mul(m2[:], G[0][:, sl], G[2][:, sl])
            nc.vector.tensor_add(cn[:, sl], m1[:], m2[:])

        def update_h(cell, kh, G, cn, hn):
            sl = slice(kh * BL, (kh + 1) * BL)
            tc_ = tmp.tile([128, BL], DT_BF, tag=f"tc{kh}", name="tcn")
            nc.scalar.activation(tc_[:], cn[:, sl], AF.Tanh)
            # hn split per batch chunk: downstream DoubleRow matmuls need both
            # feature halves but only one 512-col chunk at a time
            for n in range(2):
                cs = slice(kh * BL + n * N5, kh * BL + (n + 1) * N5)
                nc.vector.tensor_mul(hn[:, cs], G[3][:, cs], tc_[:, n * N5 : (n + 1) * N5])

        def emit_z(t, h1t):
            for n in range(2):
                nc.tensor.matmul(
                    zps[:, n * N5 : (n + 1) * N5],
                    w3(DW8, t, q=32),
                    mv3(h1t, n),
                    start=(t == 0),
                    stop=(t == D - 1),
                    skip_group_check=True,
                    perf_mode=mybir.MatmulPerfMode.DoubleRow,
                )

        # ---------- recurrent loop ----------
        prev_z = None  # deferred z so PE never waits on h1n
        for t in range(D):
            stg = stgp.tile([3, BL], DT_BF, tag="stg", name="stg")
            nc.sync.dma_start(stg[:], ohd[3 * t : 3 * t + 3, :])

            # ----- cell 0: group = [pre0-inject, onehot, hh0 x4]; ACT drains.
            # Half-A m-tiles first so h0n half A is ready while PE runs half B.
            h0t = st_tiles["h0"]
            G0 = [None] * 4
            for g in range(4):
                G0[g] = gates.tile([128, 2 * BL], DT_BF, tag=f"g{g}", name=f"g0_{g}")
            c0n = state.tile([128, 2 * BL], DT_BF, tag="c0", name="c0n")
            h0nt = state.tile([128, 2 * BL], DT_F8, tag="h0", name="h0nt")
            for mi, m in enumerate(MORDER):
                g, hf = m // 2, m % 2
                pt = ps.tile([128, BL], DT_F32, tag="g")
                for n in range(2):
                    nc.tensor.matmul(
                        pt[:, n * N5 : (n + 1) * N5],
                        wt[:, IDN : IDN + 128],
                        pre0[:, m * BL + n * N5 : m * BL + (n + 1) * N5],
                        start=True,
                        stop=False,
                    )
                for n in range(2):
                    nc.tensor.matmul(
                        pt[:, n * N5 : (n + 1) * N5],
                        wt[0:3, OHW + m * 128 : OHW + (m + 1) * 128],
                        stg[0:3, n * N5 : (n + 1) * N5],
                        start=False,
                        stop=False,
                    )
                for n in range(2):
                    nc.tensor.matmul(
                        pt[:, n * N5 : (n + 1) * N5],
                        w3(H0D, m),
                        mv3(h0t, n),
                        start=False,
                        stop=True,
                        perf_mode=mybir.MatmulPerfMode.DoubleRow,
                    )
                nc.scalar.activation(
                    G0[g][:, hf * BL : (hf + 1) * BL], pt[:],
                    AF.Tanh if g == 2 else AF.Sigmoid, scale=DESCALE,
                )
                if mi == 1 and prev_z is not None:
                    emit_z(*prev_z)
                if mi == 2:
                    update_c(0, 0, G0, c0n)
                elif mi == 3:
                    update_h(0, 0, G0, c0n, h0nt)
                elif mi == 6:
                    update_c(0, 1, G0, c0n)
                elif mi == 7:
                    update_h(0, 1, G0, c0n, h0nt)
            st_tiles["c0"] = c0n
            st_tiles["h0"] = h0nt

            # ----- cell 1: hh1 (old h1) first, then ih1 (fresh h0n)
            h0n, h1t = st_tiles["h0"], st_tiles["h1"]
            G1 = [None] * 4
            for g in range(4):
                G1[g] = gates.tile([128, 2 * BL], DT_BF, tag=f"g{g}", name=f"g1_{g}")
            c1n = state.tile([128, 2 * BL], DT_BF, tag="c1", name="c1n")
            h1nt = state.tile([128, 2 * BL], DT_F8, tag="h1", name="h1nt")
            for mi, m in enumerate(MORDER):
                g, hf = m // 2, m % 2
                pt = ps.tile([128, BL], DT_F32, tag="g")
                for n in range(2):
                    nc.tensor.matmul(
                        pt[:, n * N5 : (n + 1) * N5],
                        w3(H1D, m),
                        mv3(h1t, n),
                        start=True,
                        stop=False,
                        perf_mode=mybir.MatmulPerfMode.DoubleRow,
                    )
                for n in range(2):
                    nc.tensor.matmul(
                        pt[:, n * N5 : (n + 1) * N5],
                        w3(I1D, m),
                        mv3(h0n, n),
                        start=False,
                        stop=True,
                        perf_mode=mybir.MatmulPerfMode.DoubleRow,
                    )
                nc.scalar.activation(
                    G1[g][:, hf * BL : (hf + 1) * BL], pt[:],
                    AF.Tanh if g == 2 else AF.Sigmoid,
                    bias=msc[:, 2 + m : 3 + m], scale=DESCALE,
                )
                if mi == 2:
                    update_c(1, 0, G1, c1n)
                elif mi == 3:
                    update_h(1, 0, G1, c1n, h1nt)
                elif mi == 6:
                    update_c(1, 1, G1, c1n)
                elif mi == 7:
                    update_h(1, 1, G1, c1n, h1nt)
            st_tiles["c1"] = c1n
            st_tiles["h1"] = h1nt
            prev_z = (t, st_tiles["h1"])

        emit_z(*prev_z)
        lctx.close()  # free loop SBUF before the epilogue pool allocates

        # ---------- epilogue ----------
        with tc.tile_pool(name="ep", bufs=1) as ep:
            a1t = ep.tile([D, BL], DT_F32, tag="a1t")
            nc.gpsimd.dma_start(a1t[:], a1[:])
            z = ep.tile([D, BL], DT_F32, tag="z")
            ez = ep.tile([D, BL], DT_F32, tag="ez")
            fz = ep.tile([D, BL], DT_F32, tag="fz")
            sp = ep.tile([D, BL], DT_F32, tag="sp")
            rf = ep.tile([D, BL], DT_F32, tag="rf")
            sg = ep.tile([D, BL], DT_F32, tag="sg")
            zsg = ep.tile([D, BL], DT_F32, tag="zsg")
            pe_f = ep.tile([D, BL], DT_F32, tag="pe_f")
            pe_b = ep.tile([D, BL], DT_BF, tag="pe_b")
            az = ep.tile([D, BL], DT_F32, tag="az")
            lp_b = ep.tile([D, BL], DT_BF, tag="lp_b")
            res = ep.tile([33, BL], DT_F32, tag="res")
            res2 = ep.tile([1, BL], DT_F32, tag="res2")
            for n in range(2):
                cs = slice(n * N5, (n + 1) * N5)
                # softplus(z) = ln(1+exp(z)); sigmoid(z) = exp(z)/(1+exp(z))
                if DESCALE == 1.0:
                    nc.vector.tensor_scalar_add(z[:, cs], zps[:, cs], msc[0:D, 10:11])
                else:
                    nc.vector.tensor_scalar(
                        z[:, cs], zps[:, cs], DESCALE, msc[0:D, 10:11],
                        mybir.AluOpType.mult, mybir.AluOpType.add,
                    )
                nc.scalar.activation(ez[:, cs], z[:, cs], AF.Exp)
                nc.scalar.add(fz[:, cs], ez[:, cs], 1.0)
                nc.scalar.activation(sp[:, cs], fz[:, cs], AF.Ln)
                nc.vector.reciprocal(rf[:, cs], fz[:, cs])
                nc.vector.tensor_mul(sg[:, cs], ez[:, cs], rf[:, cs])
                nc.vector.tensor_mul(zsg[:, cs], z[:, cs], sg[:, cs])
                nc.vector.tensor_sub(pe_f[:, cs], sp[:, cs], zsg[:, cs])
                nc.vector.tensor_copy(pe_b[:, cs], pe_f[:, cs])
                nc.vector.tensor_mul(az[:, cs], a1t[:, cs], z[:, cs])
                nc.vector.tensor_sub(lp_b[:, cs], az[:, cs], sp[:, cs])
                ptp = ps.tile([128, BL], DT_F32, tag="g", name=f"ptp{n}")
                nc.tensor.matmul(
                    ptp[0:33, 0:N5],
                    wt[0:32, TRI : TRI + 33],
                    lp_b[:, cs],
                    start=True,
                    stop=True,
                )
                nc.scalar.activation(res[0:33, cs], ptp[0:33, 0:N5], AF.Copy)
                pte = ps.tile([128, BL], DT_F32, tag="g", name=f"pte{n}")
                nc.tensor.matmul(
                    pte[0:1, 0:N5],
                    wt[0:32, TRI + 33 : TRI + 34],
                    pe_b[:, cs],
                    start=True,
                    stop=True,
                )
                nc.scalar.activation(res2[0:1, cs], pte[0:1, 0:N5], AF.Copy)

            nc.sync.dma_start(out[0:D, :], res[0:D, :])          # prefix_lps.T
            nc.sync.dma_start(out[D : 2 * D, :], pe_f[:])        # per_ent.T
            nc.sync.dma_start(out[2 * D : 2 * D + 1, :], res[D : D + 1, :])  # tot_lp
            nc.sync.dma_start(out[2 * D + 1 : 2 * D + 2, :], res2[0:1, :])   # tot_ent

    _split_multi_waits(nc)
    return nc


def prep_inputs(inputs):
    f32 = np.float32
    states = np.asarray(inputs["states"], f32)
    actions = np.asarray(inputs["actions"]).astype(np.int32)
    se_W = np.asarray(inputs["se_W"], f32)
    se_b = np.asarray(inputs["se_b"], f32)
    aemb = np.asarray(inputs["aemb"], f32)
    W_ih0 = np.asarray(inputs["W_ih0"], f32)
    W_hh0 = np.asarray(inputs["W_hh0"], f32)
    b_ih0 = np.asarray(inputs["b_ih0"], f32)
    b_hh0 = np.asarray(inputs["b_hh0"], f32)
    W_ih1 = np.asarray(inputs["W_ih1"], f32)
    W_hh1 = np.asarray(inputs["W_hh1"], f32)
    b_ih1 = np.asarray(inputs["b_ih1"], f32)
    b_hh1 = np.asarray(inputs["b_hh1"], f32)
    heads_W = np.asarray(inputs["heads_W"], f32)
    heads_b = np.asarray(inputs["heads_b"], f32)

    W = np.zeros((128, WCOLS), f32)

    def put(off, kt, mt, stride, mat):
        for k in range(kt):
            for m in range(mt):
                c = off + (k * stride + m) * 128
                W[:, c : c + 128] = mat[k * 128 : (k + 1) * 128, m * 128 : (m + 1) * 128]

    put(SE, 2, 2, 2, se_W.T)
    # IH0 scaled x16: pre0 lives at 16x so it matches the fp8 (4x * 4x)
    # scaled hh0 contributions; gate ACT descales by 1/16.
    put(IH0, 2, 8, 8, (FP8_SCALE * FP8_SCALE) * W_ih0[:, :H].T)
    # one-hot stationary rows (x16): v_j = W_ih0[:, H:] @ aemb[j] + b0
    vs = aemb @ W_ih0[:, H:].T + (b_ih0 + b_hh0)[None, :]  # [3, 1024]
    W[0:3, OHW : OHW + 1024] = (FP8_SCALE * FP8_SCALE) * vs
    W[0:32, TRI : TRI + 32] = np.triu(np.ones((32, 32), f32), k=1)
    W[0:32, TRI + 32] = 1.0
    W[0:32, TRI + 33] = 1.0
    W[:, IDN : IDN + 128] = np.eye(128, dtype=f32)
    wstack = W.astype(BF)

    # fp8 stack: x4-scaled weights, m-major [k0 | k1] blocks per m-tile
    W8 = np.zeros((128, W8COLS), f32)

    def put8(off, mat, q=128):
        # mat [256, M]: m-tile blocks [k0 128cols | k1 128cols]
        mt = mat.shape[1] // q
        for m in range(mt):
            for k in range(2):
                c = off + m * 2 * q + k * q
                W8[:, c : c + q] = mat[k * 128 : (k + 1) * 128, m * q : (m + 1) * q]

    put8(H0D, FP8_SCALE * W_hh0.T)
    put8(I1D, FP8_SCALE * W_ih1.T)
    put8(H1D, FP8_SCALE * W_hh1.T)
    dW = heads_W[:, :, 1] - heads_W[:, :, 0]  # [32, 256]
    dstack = np.zeros((256, D * 32), f32)
    for t in range(D):
        dstack[:, t * 32 + t] = FP8_SCALE * dW[t]
    put8(DW8, dstack, q=32)
    wstack8 = W8.astype(NP_F8)

    misc = np.zeros((128, 16), f32)
    misc[:, 0] = se_b[:128]
    misc[:, 1] = se_b[128:]
    b1 = b_ih1 + b_hh1
    for m in range(8):
        misc[:, 2 + m] = b1[m * 128 : (m + 1) * 128]
    misc[0:D, 10] = heads_b[:, 1] - heads_b[:, 0]

    in_maps = []
    for c in range(NCORES):
        sl = slice(c * BL, (c + 1) * BL)
        st = states[sl].T  # [256, 1024]
        sTv = np.concatenate([st[:128], st[128:]], axis=1).astype(BF)
        oh = np.zeros((3 * D, BL), f32)
        oh[2, :] = 1.0  # t=0: start token row
        for t in range(1, D):
            ap = actions[sl, t - 1].astype(f32)
            oh[3 * t + 0] = 1.0 - ap
            oh[3 * t + 1] = ap
        a1v = actions[sl].T.astype(f32)
        in_maps.append(
            {
                "wstack": wstack,
                "wstack8": wstack8,
                "misc": misc,
                "sT": sTv,
                "ohd": oh.astype(BF),
                "a1": np.ascontiguousarray(a1v),
            }
        )
    return in_maps


_CACHED_NC = None


def run(inputs, trace=False):
    global _CACHED_NC
    in_maps = prep_inputs(inputs)
    if _CACHED_NC is None:
        _CACHED_NC = build_nc()
    res = None
    for attempt in range(3):
        try:
            res = run_bass_kernel_spmd(
                _CACHED_NC, in_maps, core_ids=list(range(NCORES)), trace=trace
            )
            break
        except Exception:
            if attempt == 2:
                raise
    assert res is not None
    tot_lp = np.empty((B,), np.float32)
    tot_ent = np.empty((B,), np.float32)
    prefix = np.empty((B, D), np.float32)
    per_ent = np.empty((B, D), np.float32)
    for c in range(NCORES):
        o = res.results[c]["out"]
        sl = slice(c * BL, (c + 1) * BL)
        prefix[sl] = o[0:D].T
        per_ent[sl] = o[D : 2 * D].T
        tot_lp[sl] = o[2 * D]
        tot_ent[sl] = o[2 * D + 1]
    return (tot_lp, tot_ent, prefix, per_ent), res


def kernel(**inputs):
    out, _ = run(inputs, trace=False)
    return out
